# revision 46
# baseline (speedup 1.0000x reference)
"""Trainium2 Bass kernel for nn_DictionaryLearningTokenized (vq_codebook OMP).

Self-contained: hardcodes all shapes/sharding. Data-parallel over the 16384
signals across 8 NeuronCores; the dictionary is replicated.

Algorithm (exact-arithmetic-equivalent reformulation of the reference's
batched OMP with incremental Cholesky):
  For each signal x (64-dim), K=4 steps of OMP over N=512 unit-norm atoms.
  Maintain the residual r and a Gram-Schmidt basis q_1..q_3 of the selected
  atoms, normalized reference-style (corner_k = sqrt(clip(1 - sum_j w_j^2)))
  so that the inner products w_j = <q_j, a_k> ARE the reference's Cholesky
  entries L[k,j], corner_k = L[k,k], and y_k = <a_k, r_{k-1}>/corner_k is the
  reference's forward-substitution solution. Selection uses
  argmax |D^T r| == argmax |h_bar - G x| (identical in exact arithmetic).
  Final coefficients via batched back-substitution of L^T x = y, then
  33-bin quantization, tokens, reconstruction and sq-error partials.

Per core: 2048 signals = 16 tiles of 128 (signals on SBUF partitions),
processed per OMP step in 4 pipelined groups of 4 tiles (group g's batched
inner-product/update tail overlaps group g+1's per-tile argmax chains).
  PE     : h = D^T r matmuls (N=512) + residual transposes
  ACT    : |h|, PSUM evacuation, scaled copies (q_k, recon)
  DVE    : max8/max_index (argmax), stacked inner products, fused
           scalar_tensor_tensor updates, batched small algebra
  SWDGE  : per-signal atom-row gathers from D^T by argmax index
"""

import numpy as np

P = 128          # SBUF partitions / signals per tile
M = 64           # signal dim
N = 512          # atoms
K = 4            # sparsity
T = 16           # tiles per core
BC = P * T       # signals per core (2048)
NCORES = 8
B_TOT = BC * NCORES
NBINS = 33
CMAX = 2.0
GROUPS = 4

_CACHE = {}


def _build_program(debug=False):
    import concourse.bacc as bacc
    import concourse.bass as bass
    import concourse.mybir as mybir
    import concourse.tile as tile
    from concourse.bass import IndirectOffsetOnAxis
    from concourse.masks import make_identity

    f32 = mybir.dt.float32
    i32 = mybir.dt.int32
    u32 = mybir.dt.uint32
    ALU = mybir.AluOpType
    ACT = mybir.ActivationFunctionType

    nc = bacc.Bacc("TRN2", target_bir_lowering=False, debug=False)

    # DRAM I/O (per core; same names on every core, different data per in_map)
    xs = nc.dram_tensor("xs", [T, M, P], f32, kind="ExternalInput")      # X shard, per-tile m-major blocks
    xst = nc.dram_tensor("xst", [P, T, M], f32, kind="ExternalInput")    # X shard, partition-major
    dmat = nc.dram_tensor("dmat", [M, N], f32, kind="ExternalInput")     # normalized D
    dtr = nc.dram_tensor("dtr", [N, M], f32, kind="ExternalInput")       # D^T (gather rows)
    recon_d = nc.dram_tensor("recon", [BC, M], f32, kind="ExternalOutput")
    tokf_d = nc.dram_tensor("tokf", [P, K * T], f32, kind="ExternalOutput")
    lossp_d = nc.dram_tensor("lossp", [P, 2], f32, kind="ExternalOutput")
    if debug:
        dbg = {
            "d_xc": nc.dram_tensor("d_xc", [P, K * T], f32, kind="ExternalOutput"),
            "d_ip4": nc.dram_tensor("d_ip4", [P, T * 4], f32, kind="ExternalOutput"),
            "d_ip3": nc.dram_tensor("d_ip3", [P, T * 3], f32, kind="ExternalOutput"),
            "d_yt": nc.dram_tensor("d_yt", [P, K * T], f32, kind="ExternalOutput"),
            "d_rc": nc.dram_tensor("d_rc", [P, K * T], f32, kind="ExternalOutput"),
            "d_aall": nc.dram_tensor("d_aall", [P, T * 4 * M], f32, kind="ExternalOutput"),
            "d_qr": nc.dram_tensor("d_qr", [P, T * 4 * M], f32, kind="ExternalOutput"),
        }

    with tile.TileContext(nc) as tc:
        with (
            tc.tile_pool(name="const", bufs=1) as cp,
            tc.tile_pool(name="xmp", bufs=6) as xmp,
            tc.tile_pool(name="rxp", bufs=6) as rxp,
            tc.tile_pool(name="habsp", bufs=8) as habsp,
            tc.tile_pool(name="prodp", bufs=4) as prodp,
            tc.tile_pool(name="maxvp", bufs=12) as maxvp,
            tc.tile_pool(name="up", bufs=4) as up,
            tc.tile_pool(name="zp", bufs=4) as zp,
            tc.tile_pool(name="hps_pool", bufs=4, space="PSUM") as hpsp,
            tc.tile_pool(name="tps_pool", bufs=3, space="PSUM") as tpsp,
        ):
            # ---- persistent tiles ----
            d_sb = cp.tile([M, N], f32, tag="d_sb")               # D [64,512]
            ident = cp.tile([P, P], f32, tag="ident")
            xtall = cp.tile([P, T * M], f32, tag="xtall")         # XT per tile
            qrall = cp.tile([P, T * 4 * M], f32, tag="qrall")     # [r|q1|q2|q3] per tile
            aall = cp.tile([P, T * 4 * M], f32, tag="aall")       # [a1..a4] per tile
            idx8 = [cp.tile([P, T * 8], u32, tag=f"idx8_{k}", name=f"idx8_{k}")
                    for k in range(K)]
            ip = [cp.tile([P, T * (k + 1)], f32, tag=f"ip_{k}", name=f"ip_{k}")
                  for k in range(K)]
            yt = cp.tile([P, K * T], f32, tag="yt")               # y_k  (k-major)
            rc = cp.tile([P, K * T], f32, tag="rc")               # 1/corner_k
            s16 = cp.tile([P, T], f32, tag="s16")
            corn = cp.tile([P, T], f32, tag="corn")
            wsq = cp.tile([P, T * (K - 1)], f32, tag="wsq")
            xc = cp.tile([P, K * T], f32, tag="xc")               # coefficients
            t1 = cp.tile([P, T], f32, tag="t1")
            t2 = cp.tile([P, T], f32, tag="t2")
            binf = cp.tile([P, K * T], f32, tag="binf")
            cq = cp.tile([P, K * T], f32, tag="cq")
            idxf = cp.tile([P, K * T], f32, tag="idxf")
            tokf = cp.tile([P, K * T], f32, tag="tokf")
            lp = cp.tile([P, T], f32, tag="lp")

            make_identity(nc, ident[:])
            nc.vector.memset(rc[:, 0:T], 1.0)

            # ---- loads (xtall deferred to the loss section; r-slot loads
            # are issued per group inside step 1) ----
            nc.sync.dma_start(d_sb[:], dmat[:])
            src_pt = xst[:]                                        # [128,16,64]

            def qr_slot(t, s):  # s=0 -> r, s=1..3 -> q_s
                return qrall[:, t * 4 * M + s * M: t * 4 * M + (s + 1) * M]

            # ================= OMP steps =================
            qr4 = qrall[:].rearrange("p (t s m) -> p t s m", s=4, m=M)
            a4 = aall[:].rearrange("p (t s m) -> p t s m", s=4, m=M)
            a3 = aall[:].rearrange("p (t s) -> p t s", s=4 * M)
            xt3 = xtall[:].rearrange("p (t m) -> p t m", m=M)

            def slot(v4, s, t0=0, nt=T):  # [128,nt,64] view of slot s
                return v4[:, t0:t0 + nt, s: s + 1, :].squeeze()

            def bcast(ap_pt, nt):  # [128,nt] -> [128,nt,M] broadcast
                return ap_pt.unsqueeze(2).to_broadcast([P, nt, M])

            TG = T // GROUPS

            def emit_passA(k, g):
                kk = k - 1
                idx8k = idx8[kk]
                t0 = g * TG
                if k == 1:
                    # load this group's residual slots (r = x)
                    nc.sync.dma_start(
                        qrall[:].rearrange("p (t s) -> p t s", s=4 * M)[
                            :, t0:t0 + TG, 0:M],
                        src_pt[:, t0:t0 + TG, :])
                for t in range(t0, t0 + TG):
                    hps = hpsp.tile([P, N], f32, tag="hps")
                    if k == 1:
                        xm = xmp.tile([M, P], f32, tag="xm")
                        nc.sync.dma_start(xm[:], xs[t, :, :])
                        nc.tensor.matmul(hps[:], lhsT=xm[:], rhs=d_sb[:],
                                         start=True, stop=True)
                    else:
                        rps = tpsp.tile([M, P], f32, tag="rps")
                        nc.tensor.transpose(rps[:], qr_slot(t, 0), ident[:])
                        rx = rxp.tile([M, P], f32, tag="rx")
                        nc.scalar.copy(rx[:], rps[:])
                        nc.tensor.matmul(hps[:], lhsT=rx[:], rhs=d_sb[:],
                                         start=True, stop=True)
                    habs = habsp.tile([P, N], f32, tag="habs")
                    nc.scalar.activation(habs[:], hps[:], ACT.Abs)
                    maxv = maxvp.tile([P, 8], f32, tag="maxv")
                    nc.vector.max(out=maxv[:], in_=habs[:])
                    nc.vector.max_index(out=idx8k[:, t * 8:(t + 1) * 8],
                                        in_max=maxv[:], in_values=habs[:])
                    nc.gpsimd.indirect_dma_start(
                        out=aall[:, t * 4 * M + kk * M: t * 4 * M + k * M],
                        out_offset=None,
                        in_=dtr[:],
                        in_offset=IndirectOffsetOnAxis(
                            ap=idx8k[:, t * 8: t * 8 + 1], axis=0),
                    )

            def emit_tail(k, g):
                kk = k - 1
                ipk = ip[kk]
                t0 = g * TG
                # ---- batched stacked inner products ----
                prod = prodp.tile([P, TG * k * M], f32, tag="prod")
                prod4 = prod[:].rearrange("p (t s m) -> p t s m", s=k, m=M)
                # IP multiply on GPSIMD (hidden behind the next group's argmax
                # scans) except for the final tails, where no later pass-A
                # work remains to cover GPSIMD's slower rate
                ip_eng = nc.vector if (k == K and g >= GROUPS - 2) else nc.gpsimd
                ip_eng.tensor_tensor(
                    out=prod4, in0=qr4[:, t0:t0 + TG, 0:k, :],
                    in1=a4[:, t0:t0 + TG, kk:k, :]
                    .to_broadcast([P, TG, k, M]),
                    op=ALU.mult)
                nc.vector.tensor_reduce(
                    out=ipk[:, t0 * k:(t0 + TG) * k], in_=prod4,
                    axis=mybir.AxisListType.X, op=ALU.add)

                # ---- batched small algebra ----
                ipk3 = ipk[:, t0 * k:(t0 + TG) * k].rearrange(
                    "p (t k) -> p t k", k=k)
                ysl = slice(kk * T + t0, kk * T + t0 + TG)
                if k == 1:
                    nc.vector.tensor_copy(yt[:, ysl], ipk[:, t0:t0 + TG])
                else:
                    wv = ipk3[:, :, 1:k]
                    wsqv = wsq[:, t0 * (k - 1):(t0 + TG) * (k - 1)] \
                        .rearrange("p (t j) -> p t j", j=k - 1)
                    nc.vector.tensor_tensor(out=wsqv, in0=wv, in1=wv,
                                            op=ALU.mult)
                    nc.vector.tensor_reduce(out=s16[:, t0:t0 + TG],
                                            in_=wsqv,
                                            axis=mybir.AxisListType.X,
                                            op=ALU.add)
                    nc.vector.tensor_scalar(out=s16[:, t0:t0 + TG],
                                            in0=s16[:, t0:t0 + TG],
                                            scalar1=float(1.0 - 1e-12),
                                            scalar2=None, op0=ALU.min)
                    nc.scalar.activation(corn[:, t0:t0 + TG],
                                         s16[:, t0:t0 + TG], ACT.Sqrt,
                                         bias=1.0, scale=-1.0)
                    nc.vector.reciprocal(rc[:, ysl], corn[:, t0:t0 + TG])
                    nc.vector.tensor_tensor(
                        out=yt[:, ysl], in0=ipk3[:, :, 0:1].squeeze(),
                        in1=rc[:, ysl], op=ALU.mult)

                # ---- batched basis + residual updates (skip at k=K) ----
                if k == K:
                    return
                if k == 1:
                    nc.vector.tensor_copy(out=slot(qr4, 1, t0, TG),
                                          in_=slot(a4, 0, t0, TG))
                else:
                    u = up.tile([P, TG * M], f32, tag="u")
                    u3 = u[:].rearrange("p (t m) -> p t m", m=M)
                    tmp = zp.tile([P, TG * M], f32, tag="tmp")
                    tmp3 = tmp[:].rearrange("p (t m) -> p t m", m=M)
                    for j in range(1, k):
                        nc.vector.tensor_tensor(
                            out=tmp3, in0=slot(qr4, j, t0, TG),
                            in1=bcast(ipk3[:, :, j: j + 1].squeeze(), TG),
                            op=ALU.mult)
                        nc.vector.tensor_tensor(
                            out=u3,
                            in0=(slot(a4, kk, t0, TG) if j == 1 else u3),
                            in1=tmp3, op=ALU.subtract)
                    nc.vector.tensor_tensor(
                        out=slot(qr4, k, t0, TG), in0=u3,
                        in1=bcast(rc[:, ysl], TG), op=ALU.mult)
                # r -= y_k q_k
                tmp2 = zp.tile([P, TG * M], f32, tag="tmp")
                tmp23 = tmp2[:].rearrange("p (t m) -> p t m", m=M)
                nc.vector.tensor_tensor(
                    out=tmp23, in0=slot(qr4, k, t0, TG),
                    in1=bcast(yt[:, ysl], TG), op=ALU.mult)
                nc.vector.tensor_tensor(
                    out=slot(qr4, 0, t0, TG), in0=slot(qr4, 0, t0, TG),
                    in1=tmp23, op=ALU.subtract)

            # software-pipelined emission: each group's DVE tail is emitted
            # AFTER the next group's argmax chains so the static per-engine
            # schedule hides gather/IP latency behind the next group's scans
            stages = [(k, g) for k in range(1, K + 1) for g in range(GROUPS)]
            emit_passA(*stages[0])
            for i in range(1, len(stages)):
                emit_passA(*stages[i])
                emit_tail(*stages[i - 1])
            emit_tail(*stages[-1])

            # ===== back-substitution / quantization / recon (2 halves) =====
            # xtall only feeds the loss; load it late so it never gates OMP
            nc.sync.dma_start(xtall[:].rearrange("p (t m) -> p t m", m=M),
                              src_pt)

            def ipvh(k, j, t0, nt):  # L[k,j] view [128,nt]
                return ip[k - 1][:, t0 * k:(t0 + nt) * k].rearrange(
                    "p (t k) -> p t k", k=k)[:, :, j:j + 1].squeeze()

            EH = T // 2  # endgame half size
            for h in range(2):
                t0 = h * EH

                def ysh(k):
                    return yt[:, (k - 1) * T + t0: (k - 1) * T + t0 + EH]

                def rch(k):
                    return rc[:, (k - 1) * T + t0: (k - 1) * T + t0 + EH]

                def xch(k):
                    return xc[:, (k - 1) * T + t0: (k - 1) * T + t0 + EH]

                t1h = t1[:, t0:t0 + EH]
                t2h = t2[:, t0:t0 + EH]
                TT = nc.vector.tensor_tensor
                # x4 = y4 * rc4
                TT(out=xch(4), in0=ysh(4), in1=rch(4), op=ALU.mult)
                # x3 = (y3 - L43 x4) rc3
                TT(out=t1h, in0=ipvh(4, 3, t0, EH), in1=xch(4), op=ALU.mult)
                TT(out=t1h, in0=ysh(3), in1=t1h, op=ALU.subtract)
                TT(out=xch(3), in0=t1h, in1=rch(3), op=ALU.mult)
                # x2 = (y2 - L32 x3 - L42 x4) rc2
                TT(out=t1h, in0=ipvh(3, 2, t0, EH), in1=xch(3), op=ALU.mult)
                TT(out=t2h, in0=ipvh(4, 2, t0, EH), in1=xch(4), op=ALU.mult)
                TT(out=t1h, in0=t1h, in1=t2h, op=ALU.add)
                TT(out=t1h, in0=ysh(2), in1=t1h, op=ALU.subtract)
                TT(out=xch(2), in0=t1h, in1=rch(2), op=ALU.mult)
                # x1 = y1 - L21 x2 - L31 x3 - L41 x4
                TT(out=t1h, in0=ipvh(2, 1, t0, EH), in1=xch(2), op=ALU.mult)
                TT(out=t2h, in0=ipvh(3, 1, t0, EH), in1=xch(3), op=ALU.mult)
                TT(out=t1h, in0=t1h, in1=t2h, op=ALU.add)
                TT(out=t2h, in0=ipvh(4, 1, t0, EH), in1=xch(4), op=ALU.mult)
                TT(out=t1h, in0=t1h, in1=t2h, op=ALU.add)
                TT(out=xch(1), in0=ysh(1), in1=t1h, op=ALU.subtract)

                # quantization: bin = RNE-round(8c+16) via 2^23+2^22 magic
                xcv = xc[:].rearrange("p (k t) -> p k t", t=T)[:, :, t0:t0 + EH]
                bfv = binf[:].rearrange("p (k t) -> p k t", t=T)[:, :, t0:t0 + EH]
                cqv = cq[:].rearrange("p (k t) -> p k t", t=T)[:, :, t0:t0 + EH]
                nc.vector.tensor_scalar(out=xcv, in0=xcv, scalar1=CMAX,
                                        scalar2=-CMAX, op0=ALU.min, op1=ALU.max)
                nc.scalar.activation(bfv, xcv, ACT.Copy, bias=16.0, scale=8.0)
                nc.vector.tensor_scalar_add(bfv, bfv, 12582912.0)
                nc.vector.tensor_scalar_add(bfv, bfv, -12582912.0)
                nc.scalar.activation(cqv, bfv, ACT.Copy, bias=-2.0, scale=0.125)

                # reconstruction + loss for this half
                z = zp.tile([P, EH * M], f32, tag="z")
                z3 = z[:].rearrange("p (t m) -> p t m", m=M)
                tmpz = up.tile([P, EH * M], f32, tag="u")
                tmpz3 = tmpz[:].rearrange("p (t m) -> p t m", m=M)
                for k in range(1, K + 1):
                    cqb = cq[:, (k - 1) * T + t0: (k - 1) * T + t0 + EH] \
                        .unsqueeze(2).to_broadcast([P, EH, M])
                    if k == 1:
                        nc.vector.tensor_tensor(
                            out=z3, in0=slot(a4, 0, t0, EH), in1=cqb,
                            op=ALU.mult)
                    else:
                        nc.vector.tensor_tensor(
                            out=tmpz3, in0=slot(a4, k - 1, t0, EH), in1=cqb,
                            op=ALU.mult)
                        nc.vector.tensor_tensor(out=z3, in0=z3, in1=tmpz3,
                                                op=ALU.add)
                nc.sync.dma_start(
                    recon_d[t0 * P:(t0 + EH) * P, :]
                    .rearrange("(t p) m -> p t m", p=P), z3)
                dif = up.tile([P, EH * M], f32, tag="u")
                dif3 = dif[:].rearrange("p (t m) -> p t m", m=M)
                nc.vector.tensor_tensor(out=dif3, in0=z3,
                                        in1=xt3[:, t0:t0 + EH, :],
                                        op=ALU.subtract)
                dsq = zp.tile([P, EH * M], f32, tag="z")
                nc.scalar.activation(dsq[:], dif[:], ACT.Square,
                                     accum_out=lp[:, h: h + 1])

            # tokens (off the critical path; reads full binf)
            for k in range(1, K + 1):
                nc.vector.tensor_copy(
                    out=idxf[:, (k - 1) * T: k * T],
                    in_=idx8[k - 1][:].rearrange(
                        "p (t e) -> p t e", e=8)[:, :, 0:1].squeeze())
            nc.vector.scalar_tensor_tensor(
                out=tokf[:], in0=idxf[:], scalar=float(NBINS), in1=binf[:],
                op0=ALU.mult, op1=ALU.add)
            nc.sync.dma_start(tokf_d[:], tokf[:])
            nc.sync.dma_start(lossp_d[:], lp[:, 0:2])
            if debug:
                nc.sync.dma_start(dbg["d_xc"][:], xc[:])
                nc.sync.dma_start(dbg["d_ip4"][:], ip[3][:])
                nc.sync.dma_start(dbg["d_ip3"][:], ip[2][:])
                nc.sync.dma_start(dbg["d_yt"][:], yt[:])
                nc.sync.dma_start(dbg["d_rc"][:], rc[:])
                nc.sync.dma_start(dbg["d_aall"][:], aall[:])
                nc.sync.dma_start(dbg["d_qr"][:], qrall[:])

    nc.compile()
    return nc


def _get_program():
    if "nc" not in _CACHE:
        _CACHE["nc"] = _build_program()
    return _CACHE["nc"]


def _host_prep(z_e, dictionary):
    f32 = np.float32
    z_e = np.asarray(z_e, dtype=f32)
    dic = np.asarray(dictionary, dtype=f32)
    nrm = np.sqrt((dic * dic).sum(axis=0, dtype=f32)).astype(f32)
    D = (dic / np.maximum(nrm, f32(1e-10))).astype(f32)
    DT = np.ascontiguousarray(D.T)
    signals = np.ascontiguousarray(
        z_e.transpose(0, 2, 3, 1).reshape(-1, M))            # [16384, 64]
    return D, DT, signals


def _shard_maps(D, DT, signals):
    in_maps = []
    for c in range(NCORES):
        sh = signals[c * BC:(c + 1) * BC]                    # [2048, 64]
        s3 = sh.reshape(T, P, M)
        in_maps.append({
            "xs": np.ascontiguousarray(s3.transpose(0, 2, 1)),   # [T, 64, 128]
            "xst": np.ascontiguousarray(s3.transpose(1, 0, 2)),  # [128, T, 64]
            "dmat": D,
            "dtr": DT,
        })
    return in_maps


def kernel(z_e, dictionary):
    from concourse.bass_utils import run_bass_kernel_spmd

    nc = _get_program()
    D, DT, signals = _host_prep(z_e, dictionary)
    in_maps = _shard_maps(D, DT, signals)
    res = run_bass_kernel_spmd(nc, in_maps, core_ids=list(range(NCORES)))

    recon = np.concatenate([res.results[c]["recon"] for c in range(NCORES)],
                           axis=0)                            # [16384, 64]
    z_q = recon.reshape(16, 32, 32, M).transpose(0, 3, 1, 2).astype(np.float32)

    toks = []
    for c in range(NCORES):
        tf = res.results[c]["tokf"]                           # [128, K*T]
        t3 = tf.reshape(P, K, T).transpose(2, 0, 1).reshape(BC, K)
        toks.append(t3)
    tokens = np.rint(np.concatenate(toks, axis=0)).astype(np.int32)
    tokens = tokens.reshape(16, 32, 32, K)

    sq = np.zeros((), np.float64)
    for c in range(NCORES):
        sq += res.results[c]["lossp"].astype(np.float64).sum()
    mse = np.float32(sq / (B_TOT * M))
    loss = np.float32(np.float32(1.25) * mse)

    z_e = np.asarray(z_e, dtype=np.float32)
    z_q_ste = z_q  # z_e + stop_grad(z_q - z_e) == z_q numerically
    return z_q_ste, loss, tokens


def timed_run(np_inputs, trace_cores=None):
    """Best-effort device timing.

    The axon client in this container has no NTFF profile hook, so a real
    neuron-profile exec time is unavailable; fall back to the cost-model
    timeline estimate (single core, all cores run identical programs).
    """
    from concourse.bass_utils import run_bass_kernel_spmd

    nc = _get_program()
    try:
        from concourse.timeline_sim import TimelineSim
        est = TimelineSim(nc, trace=False).simulate()
        return int(est)
    except Exception:
        return None
